# revision 15
# baseline (speedup 1.0000x reference)
"""Trainium2 Bass kernel for nn_LSM_IniReconNet.

The reference computes, per contiguous 16-element block of the signal,
z = W1 @ block then y = W2 @ z — i.e. a fixed 16x16 linear map
M = W2 @ W1 applied blockwise. This kernel streams the signal through
the chip: a DVE 32x32 stream-transpose puts the block offset t on SBUF
partitions (t = partition mod 16), one PE matmul against the constant
stationary K = kron(I8, M.T) applies the transform to 8 blocks of
partitions at once, and a second stream transpose (reading PSUM)
restores the layout for contiguous DMA out.

Sharding: pure data parallel — batch rows split across 8 cores, K
replicated.
"""

import sys

for _p in ("/opt/trn_rl_repo", "/root/.axon_site/_ro/trn_rl_repo"):
    if _p not in sys.path:
        sys.path.insert(0, _p)

import numpy as np

import concourse.bass as bass
import concourse.mybir as mybir
from concourse.bass_utils import run_bass_kernel_spmd
from concourse.tile import TileContext

F32 = mybir.dt.float32
F32R = mybir.dt.float32r

NB = 4096  # batch
H = 4096  # signal length
BLOCK = 16
SP = 8
N_CORES = 8
ROWS_PER_CORE = NB // N_CORES  # 512

_NC_CACHE = {}


def _split_multi_waits(nc):
    """walrus codegen accepts at most one semaphore wait per instruction
    (beyond what same-queue elision removes). Tile attaches several — most
    notably on the kernel-tail drain. Hoist all but one wait onto wait-only
    NOPs placed immediately before the instruction on the same engine queue.
    """
    ctr = 0
    for fn in nc.m.functions:
        for blk in fn.blocks:
            old = list(blk.instructions)
            if not any(
                i.sync_info is not None and len(i.sync_info.on_wait) > 1 for i in old
            ):
                continue
            new = []
            for inst in old:
                si = inst.sync_info
                if si is not None and len(si.on_wait) > 1:
                    waits = list(si.on_wait)
                    for w in waits[:-1]:
                        ctr += 1
                        new.append(
                            mybir.InstNoOp(
                                name=f"I-waitsplit-{ctr}",
                                sync_info=mybir.SyncInfo(on_wait=[w], on_update=[]),
                                bass_nofuse=True,
                                engine=inst.engine,
                            )
                        )
                    inst.sync_info = mybir.SyncInfo(
                        on_wait=[waits[-1]], on_update=list(si.on_update)
                    )
                new.append(inst)
            blk.instructions = new
    return nc


def _build(nrows, ncols):
    """Per-core SPMD program: y = blockwise-16 transform of x.

    x: (nrows, ncols) fp32; k: (128, 128) fp32 = kron(I8, M.T).
    """
    nc = bass.Bass()
    x = nc.declare_dram_parameter("x", [nrows, ncols], F32, isOutput=False)
    k = nc.declare_dram_parameter("k", [128, 128], F32, isOutput=False)
    y = nc.declare_dram_parameter("y", [nrows, ncols], F32, isOutput=True)
    ngroups = nrows // 128  # 128-row groups
    npairs = ngroups // 2  # one input DMA covers two groups
    nslices = ncols // 512

    # Constraint (walrus codegen): DMA-copy and matmul instructions carry at
    # most ONE sync wait; DVE ops at most two. Structure below keeps within
    # that: <=8 DMAs total (no DMA-lane semaphore reuse), yout pool sized so
    # output tiles are never recycled, K consumed by a warm-up matmul.
    with TileContext(nc) as tc:
        with (
            tc.tile_pool(name="kpool", bufs=1) as kp,
            tc.tile_pool(name="xin", bufs=4) as xin,
            tc.tile_pool(name="t1", bufs=4) as t1p,
            tc.tile_pool(name="yout", bufs=6) as yp,
            tc.tile_pool(name="ps", bufs=8, space="PSUM") as pp,
        ):
            k_sb = kp.tile([128, 128], F32)
            nc.gpsimd.dma_start(out=k_sb[:], in_=k[:])
            # Warm-up matmul: consumes the K-DMA wait early.
            ps = pp.tile([128, 512], F32, tag="ps")
            nc.tensor.matmul(ps[:, :128], k_sb[:], k_sb[:], start=True, stop=True)
            half = ncols // 2
            for g in range(ngroups):
                rows = slice(g * 128, (g + 1) * 128)
                for h in range(2):
                    cols = slice(h * half, (h + 1) * half)
                    xt = xin.tile([128, half], F32)
                    nc.gpsimd.dma_start(out=xt[:], in_=x[rows, cols])
                    t1 = t1p.tile([128, half], F32)
                    nc.vector.transpose(t1[:], xt[:])
                    yt = yp.tile([128, half], F32)
                    for s in range(half // 512):
                        ps = pp.tile([128, 512], F32, tag="ps")
                        nc.tensor.matmul(
                            ps[:],
                            k_sb[:],
                            t1[:, s * 512 : (s + 1) * 512],
                            start=True,
                            stop=True,
                        )
                        nc.vector.transpose(yt[:, s * 512 : (s + 1) * 512], ps[:])
                    nc.gpsimd.dma_start(out=y[rows, cols], in_=yt[:])
    return _split_multi_waits(nc)


def _get_nc():
    key = (ROWS_PER_CORE, H)
    if key not in _NC_CACHE:
        _NC_CACHE[key] = _build(*key)
    return _NC_CACHE[key]


def _run(x, W_samp, W_init, **run_kwargs):
    x2d = np.ascontiguousarray(np.asarray(x, dtype=np.float32).reshape(NB, H))
    W1 = np.asarray(W_samp, dtype=np.float32)[:, 0, :]  # (8, 16)
    W2 = np.asarray(W_init, dtype=np.float32)[:, :, 0]  # (16, 8)
    M = W2 @ W1  # (16, 16)
    K = np.ascontiguousarray(np.kron(np.eye(SP, dtype=np.float32), M.T))

    nc = _get_nc()
    in_maps = [
        {"x": x2d[i * ROWS_PER_CORE : (i + 1) * ROWS_PER_CORE], "k": K}
        for i in range(N_CORES)
    ]
    res = run_bass_kernel_spmd(nc, in_maps, list(range(N_CORES)), **run_kwargs)
    out = np.concatenate([res.results[i]["y"] for i in range(N_CORES)], axis=0)
    return out.reshape(NB, H, 1), res


def kernel(x, W_samp, W_init):
    out, _ = _run(x, W_samp, W_init)
    return out
